# revision 4
# baseline (speedup 1.0000x reference)
"""DelayBuffer Trainium2 kernel (pure-DMA, aligned full-tile stores).

Input:  embeddings [4, 4096, 1024] f32.
Output: [4, 4096, 6144] f32 -- for each delay d in (1,2,4,8,16,32),
        out_d[t] = emb[t-d] if t >= d else emb[t], concatenated on the
        last axis.

Sharding: 8 cores = (batch b in 0..3) x (embed half h in 0..1); each
core handles a contiguous [4096, 512] shard (8MB).

Kernel (per core):
  1. one 8MB load into a [128, 16384] SBUF tile (row r -> partition
     r//32, chunk r%32, so partition-major order equals row order);
  2. per delay d: ONE full-tile 8MB store of the shifted copy plus one
     tiny head store. The full store writes x[t-d] to logical row t for
     ALL t in [d, 4095+d]; the d trailing rows spill into a 64KB pad
     region at the end of each output slot and are ignored by the host.
The six big stores are spread over the three DMA queues (gpsimd/SWDGE,
sync and scalar/HWDGE) 2-2-2; load and heads stay on gpsimd.

Output layout: y = flat [6 * SLOT] f32, SLOT = 4096*512 + 16384. The
logical rows 0..4095 of delay k live at o_k = k*SLOT + (16384 - d_k*512)
% 16384, which makes every big store destination (o_k + d_k*512) 64KB-
aligned. Aligned 128-partition full-tile stores are the fast DMA shape
on this runtime (~18.7us per 8MB sustained vs ~68us for the offset
127-partition stores the previous version used).

Host reassembles: out[b, :, k*D+h*C : ...] = y[o_k : o_k + S*C].
"""

import numpy as np

import concourse.bass as bass
import concourse.tile as tile
from concourse import mybir
from concourse.bass_utils import run_bass_kernel_spmd

DELAYS = (1, 2, 4, 8, 16, 32)
B, S, D = 4, 4096, 1024
NCORES = 8
C = 512               # channels per core (half of D)
P = 128               # SBUF partitions
RPP = S // P          # rows per partition = 32
FREE = RPP * C        # f32 per partition = 16384 (64KB)
PAD = 16384           # 64KB pad region per output slot
SLOT = S * C + PAD    # f32 per output slot
NK = len(DELAYS)

_cached_nc = None


def _offsets():
    # o_k: logical row-0 position of delay-k output within y.
    return [k * SLOT + ((PAD - d * C) % PAD) for k, d in enumerate(DELAYS)]


def _split_multi_waits(nc: bass.Bass) -> None:
    # This walrus version can encode only ONE sync-wait per instruction
    # (the TPB header's single EVENTS slot); codegen aborts with "Too many
    # sync wait commands" otherwise. The Tile kernel-tail drain waits on
    # every DMA sem lane, so split: hoist all but the last wait onto
    # fresh single-wait NoOps inserted just before the instruction on the
    # same engine queue.
    for f in nc.m.functions:
        for bb in f.blocks:
            new_insts = []
            for inst in bb.instructions:
                si = getattr(inst, "sync_info", None)
                if si is not None and si.on_wait and len(si.on_wait) > 1:
                    for w in si.on_wait[:-1]:
                        nop = mybir.InstNoOp(
                            name=nc.get_next_instruction_name(),
                            engine=inst.engine,
                        )
                        nop.sync_info = mybir.SyncInfo(on_wait=[w], on_update=[])
                        new_insts.append(nop)
                    si.on_wait = [si.on_wait[-1]]
                new_insts.append(inst)
            bb.instructions[:] = new_insts


def _build_program(reps: int = 1) -> bass.Bass:
    # reps > 1 repeats the kernel serially inside one NEFF (the shared
    # SBUF tile's WAR deps serialize reps) -- used only for marginal-reps
    # benchmarking, which cancels the multi-ms PJRT dispatch overhead.
    nc = bass.Bass()
    x = nc.declare_dram_parameter("x", [S, C], mybir.dt.float32, isOutput=False)
    y = nc.declare_dram_parameter(
        "y", [NK * SLOT], mybir.dt.float32, isOutput=True
    )
    g = nc.gpsimd
    engs = [g, nc.sync, nc.scalar, g, nc.sync, nc.scalar]
    offs = _offsets()
    with tile.TileContext(nc) as tc:
        with tc.tile_pool(name="sbuf", bufs=1) as pool:
            xt = pool.tile([P, FREE], mybir.dt.float32)
            for _ in range(reps):
                g.dma_start(
                    out=xt[:], in_=x.rearrange("(p n) c -> p n c", p=P)
                )
                for k, d in enumerate(DELAYS):
                    o = offs[k]
                    engs[k].dma_start(
                        out=y[o + d * C : o + d * C + S * C], in_=xt[:]
                    )
                    g.dma_start(out=y[o : o + d * C], in_=xt[0:1, 0 : d * C])
    _split_multi_waits(nc)
    return nc


def kernel(embeddings: np.ndarray) -> np.ndarray:
    global _cached_nc
    embeddings = np.ascontiguousarray(embeddings, dtype=np.float32)
    assert embeddings.shape == (B, S, D)

    if _cached_nc is None:
        _cached_nc = _build_program()
    nc = _cached_nc

    # Shard: core c -> batch c//2, embed half c%2.
    in_maps = []
    for c in range(NCORES):
        b, h = divmod(c, 2)
        in_maps.append(
            {"x": np.ascontiguousarray(embeddings[b, :, h * C : (h + 1) * C])}
        )

    results = run_bass_kernel_spmd(nc, in_maps, list(range(NCORES))).results

    offs = _offsets()
    out = np.empty((B, S, NK * D), dtype=np.float32)
    for c in range(NCORES):
        b, h = divmod(c, 2)
        yv = results[c]["y"]
        for k in range(NK):
            out[b, :, k * D + h * C : k * D + (h + 1) * C] = yv[
                offs[k] : offs[k] + S * C
            ].reshape(S, C)
    return out
